# revision 38
# baseline (speedup 1.0000x reference)
"""CenterLoss kernel for 8 Trainium2 NeuronCores.

loss = mean(distmat * onehot(labels)) over a (B, C) distmat where
distmat[i, j] = ||x_i - c_j||^2.  The mask selects exactly one element
per row, so  loss = (1/(B*C)) * sum_i ||x_i - c_{labels[i]}||^2.

Strategy: data-parallel over batch.  Each of the 8 cores takes 512 rows
of x, gathers its 512 center rows from the (replicated) centers table
with 4 indirect DMAs (one per 128-row chunk, pipelined against the
vector engine), computes sum((x-g)^2) per chunk via subtract +
fused square-reduce (scalar_tensor_tensor accum), and writes a [128,4]
partial-sum tile.  The host sums the partials in float64 and divides
by B*C.

Raw Bass (no Tile).  Cross-engine deps are taken with
instruction-ATTACHED semaphore waits (at most one per instruction):
an attached wait parks the decoded instruction at the sequencer and
fires within ~25-310ns of the semaphore, where a standalone wait_ge
followed by a DMA_INDIRECT costs ~0.9us of GPSIMD wake/dispatch
latency (measured; the standalone->attached change alone was -0.9us).

Measured dead ends (do not revisit without new evidence): [P,2]-offset
indirect DMAs pass CoreSim but HW honors only ONE offset per partition
(rel err 0.56); warm-up indirect DMAs cost more than the dispatch
latency they hide (~1.1us fixed emission per call regardless of
descriptor count); Q7 library ops (dma_gather/ap_gather) execute in
~0.4us but the library load is ~14us on every execution; CCE
compute_op=add inflates emission 1.15->1.78us per gather; SWDGE static
loads have ~3.2us receipts vs HWDGE ~1.4us.
"""

import sys

if "/opt/trn_rl_repo" not in sys.path:
    sys.path.insert(0, "/opt/trn_rl_repo")

import numpy as np

import concourse.bass as bass
from concourse import mybir

NCORES = 8
B = 4096
D = 128
C = 20000
P = 128
BS = B // NCORES          # 512 rows per core
N = BS // P               # 4 rows per partition


def build_bass() -> bass.Bass:
    nc = bass.Bass(num_swdge_queues=2)
    x = nc.declare_dram_parameter("x", [BS, D], mybir.dt.float32, isOutput=False)
    idx = nc.declare_dram_parameter("idx", [BS], mybir.dt.int32, isOutput=False)
    centers = nc.declare_dram_parameter(
        "centers", [C, D], mybir.dt.float32, isOutput=False
    )
    out = nc.declare_dram_parameter("out", [P, N + 2], mybir.dt.float32, isOutput=True)

    with (
        nc.sbuf_tensor([P, N], mybir.dt.int32) as idx_t,
        nc.sbuf_tensor([P, N, D], mybir.dt.float32) as x_t,
        nc.sbuf_tensor([P, N, D], mybir.dt.float32) as g_t,
        nc.sbuf_tensor([P, N, D], mybir.dt.float32) as d_t,
        nc.sbuf_tensor([P, N, D], mybir.dt.float32) as sq_t,
        nc.sbuf_tensor([P, N + 2], mybir.dt.float32) as red_t,
        nc.sbuf_tensor([P, D], mybir.dt.float32) as asq_t,
        nc.semaphore("idx_sem") as idx_sem,
        nc.semaphore("x_sem") as x_sem,
        nc.semaphore("ga_sem") as ga_sem,
        nc.semaphore("gb_sem") as gb_sem,
        nc.semaphore("gc_sem") as gc_sem,
        nc.semaphore("gd_sem") as gd_sem,
        nc.semaphore("v_sem") as v_sem,
        nc.semaphore("a_sem") as a_sem,
        nc.semaphore("done_sem") as done_sem,
    ):
        g_sems = [ga_sem, gb_sem, gc_sem, gd_sem]

        # Issue the input loads in `main`, before the Block bodies: they
        # start earlier and their completion overlaps the block entry
        # overhead.  (Kept: the Block-end barrier is load-bearing — it
        # keeps the NRT per-engine postamble from contending with
        # in-flight gather completion semaphores.)
        idx_dma = nc.sync.dma_start(
            out=idx_t[:], in_=idx[:].rearrange("(p n) -> p n", p=P)
        )
        # single_packet measured inert for 128-partition transfers
        # (walrus falls back above the per-packet descriptor limit);
        # kept because it is harmless and correctness-verified.
        idx_dma.ins.single_packet = True
        idx_dma.then_inc(idx_sem, 16)
        nc.sync.dma_start(
            out=x_t[:], in_=x[:].rearrange("(p n) d -> p n d", p=P)
        ).then_inc(x_sem, 16)

        with nc.Block(no_gpsimd_drain=True) as block:

            @block.sync
            def _(sync):
                # No wait on done_sem: the Sync queue drain at block end
                # guarantees the store lands before kernel completion.
                # The ACT accumulator flush (a_sem) is the LAST gate
                # (ACTIVATE 401ns + flush 279ns vs DVE 291+83), so it
                # gets the attached wait; the earlier DVE gate takes the
                # standalone wait whose resume latency is then hidden.
                sync.wait_ge(v_sem, 2 * N)
                out_dma = sync.dma_start(out=out[:], in_=red_t[:])
                out_dma._wait_ge(a_sem, 1)
                out_dma.ins.single_packet = True
                out_dma.then_inc(done_sem, 16)

            @block.gpsimd
            def _(gpsimd):
                # HW honors only one offset per partition per indirect
                # DMA, so issue N gathers with [P, 1] offset tiles.
                # The idx wait is ATTACHED to the first gather (one
                # attached wait per instruction is allowed): a
                # standalone wait_ge retires in ~20ns but the next
                # DMA_INDIRECT slice still starts ~0.9us later (Q7
                # wake/dispatch); an attached wait holds the decoded
                # instruction at the sequencer instead.
                for n in range(N):
                    gi = gpsimd.indirect_dma_start(
                        out=g_t[:, n, :],
                        out_offset=None,
                        in_=centers[:],
                        in_offset=bass.IndirectOffsetOnAxis(
                            ap=idx_t[:, n : n + 1], axis=0
                        ),
                    )
                    if n == 0:
                        gi._wait_ge(idx_sem, 16)
                    # alternate the two SWDGE queues so transfers overlap
                    # (single_packet on the gathers measured neutral-to-
                    # slightly-worse: +45ns emission, +64ns land)
                    if n % 2 == 1:
                        gi.ins.queue = "qPoolDynamic1"
                    gi.then_inc(g_sems[n], 16)

            @block.scalar
            def _(scalar):
                # Last chunk's sum(g^2) on the ACT engine, concurrent
                # with DVE's sum(x*g): the exposed tail after the last
                # gather lands shrinks from subtract+square (~0.65us
                # serial) to max(ACT square, DVE mult-accum).
                scalar.activation(
                    out=asq_t[:],
                    in_=g_t[:, N - 1, :],
                    func=mybir.ActivationFunctionType.Square,
                    accum_out=red_t[:, N + 1 : N + 2],
                )._wait_ge(g_sems[N - 1], 16).then_inc(a_sem, 1)

            @block.vector
            def _(vector):
                vector.wait_ge(x_sem, 16)
                # Precompute sum(x^2) of the last chunk during the idle
                # window before any gather lands (free).
                vector.scalar_tensor_tensor(
                    out=sq_t[:, N - 1, :],
                    in0=x_t[:, N - 1, :],
                    scalar=0.0,
                    in1=x_t[:, N - 1, :],
                    op0=mybir.AluOpType.add,
                    op1=mybir.AluOpType.mult,
                    accum_out=red_t[:, N : N + 1],
                ).then_inc(v_sem, 1)
                # Chunk n computes while chunk n+1's gather is in
                # flight.  All waits are instruction-attached (decoded
                # op parked at the sequencer fires on the sem) — a
                # standalone wait adds ~0.1us of resume latency per
                # retirement, which is exposed on the last chunk.  The
                # v_sem chain between dependent DVE ops keeps the race
                # detector happy.
                for n in range(N - 1):
                    vector.tensor_tensor(
                        out=d_t[:, n, :],
                        in0=x_t[:, n, :],
                        in1=g_t[:, n, :],
                        op=mybir.AluOpType.subtract,
                    )._wait_ge(g_sems[n], 16).then_inc(v_sem, 1)
                    # sq = (d + 0) * d ; accum = sum(sq) — fused
                    # square+reduce
                    vector.scalar_tensor_tensor(
                        out=sq_t[:, n, :],
                        in0=d_t[:, n, :],
                        scalar=0.0,
                        in1=d_t[:, n, :],
                        op0=mybir.AluOpType.add,
                        op1=mybir.AluOpType.mult,
                        accum_out=red_t[:, n : n + 1],
                    )._wait_ge(v_sem, 2 * n + 2).then_inc(v_sem, 1)
                # last chunk: sum(x*g) only — sum(x^2) was precomputed
                # and sum(g^2) runs on ACT.  Host combines.
                vector.scalar_tensor_tensor(
                    out=d_t[:, N - 1, :],
                    in0=g_t[:, N - 1, :],
                    scalar=1.0,
                    in1=x_t[:, N - 1, :],
                    op0=mybir.AluOpType.mult,
                    op1=mybir.AluOpType.mult,
                    accum_out=red_t[:, N - 1 : N],
                )._wait_ge(g_sems[N - 1], 16).then_inc(v_sem, 1)

    if not nc.is_finalized():
        nc.finalize()
    _hoist_idx_dma(nc)
    return nc


def _hoist_idx_dma(nc: bass.Bass) -> None:
    """Move the idx DMA above the Sync engine's preamble exit barrier.

    The Sync engine finishes its own preamble work ~0.6us before the
    all-engine barrier releases (GPSIMD's const memsets are the
    laggard).  Emitting the idx load in that idle window — after Sync's
    preamble DRAIN (which must not wait on an in-flight DMA) but before
    its barrier wait — lands idx ~0.55us earlier and shifts the whole
    gather chain left.  The DMA reads DRAM (host-written before launch)
    and writes idx_t, which nothing in the preamble touches.
    """
    eb = nc.main_func.blocks[0]
    ins_list = list(eb.instructions)
    dma_i = next(
        i
        for i, x in enumerate(ins_list)
        if type(x).__name__ == "InstDMACopy"
        and str(getattr(x, "engine", "")).endswith("SP")
    )
    # Insert before Sync's first preamble register MOVE: the emission
    # then starts ~0.33us earlier still.  The preamble-exit DRAIN now
    # blocks on the idx receipt (~8.2us) and delays the all-engine
    # barrier release — but every consumer is semaphore-parked, so the
    # only effect is idx landing earlier.
    mov_i = next(
        i
        for i, x in enumerate(ins_list)
        if type(x).__name__ == "InstRegisterMove"
        and str(getattr(x, "engine", "")).endswith("SP")
    )
    assert mov_i < dma_i, (mov_i, dma_i)
    inst = ins_list.pop(dma_i)
    ins_list.insert(mov_i, inst)
    eb.instructions = ins_list


_NC = None


def _get_nc() -> bass.Bass:
    global _NC
    if _NC is None:
        _NC = build_bass()
    return _NC


def make_in_maps(x, labels, centers):
    x = np.ascontiguousarray(np.asarray(x, dtype=np.float32))
    labels = np.asarray(labels).astype(np.int32)
    centers = np.ascontiguousarray(np.asarray(centers, dtype=np.float32))
    in_maps = []
    r = np.arange(BS)
    # sorted rank r -> device row index, matched to the SDMA engine
    # swizzle: engine k serves partitions {4k..4k+3, 64+4k..64+4k+3},
    # so give it 8 CONSECUTIVE sorted ranks (one tight HBM window per
    # engine per gather) instead of two windows 64 partitions apart.
    n_chunk, within = r // P, r % P
    eng, j = within // 8, within % 8
    part = np.where(j < 4, 4 * eng + j, 64 + 4 * eng + (j - 4))
    dev_pos = part * N + n_chunk
    for c in range(NCORES):
        sl = slice(c * BS, (c + 1) * BS)
        xs, ls = x[sl], labels[sl]
        # Sort rows by label (the loss is a sum — row order is free) and
        # place sorted rank r at device row (r%P)*N + r//P, so gather
        # chunk n reads 128 ASCENDING center addresses spanning only the
        # n-th quarter of the sorted label range: HBM-controller-friendly
        # access order instead of 128 random rows per indirect DMA.
        order = np.argsort(ls, kind="stable")
        dev = np.empty(BS, dtype=np.int64)
        dev[dev_pos] = order
        in_maps.append(
            {
                "x": np.ascontiguousarray(xs[dev]),
                "idx": np.ascontiguousarray(ls[dev]),
                "centers": centers,
            }
        )
    return in_maps


def reduce_outputs(results) -> np.ndarray:
    total = 0.0
    for r in results:
        o = r["out"].astype(np.float64)  # [P, N+2]
        # cols 0..N-2: per-chunk sum((x-g)^2); col N-1: sum(x*g) of the
        # last chunk; col N: sum(x^2); col N+1: sum(g^2)
        total += float(
            o[:, : N - 1].sum()
            + o[:, N].sum()
            + o[:, N + 1].sum()
            - 2.0 * o[:, N - 1].sum()
        )
    return np.array(np.float32(total / (B * C)))


def kernel(x, labels, centers) -> np.ndarray:
    from concourse.bass_utils import run_bass_kernel_spmd

    nc = _get_nc()
    in_maps = make_in_maps(x, labels, centers)
    res = run_bass_kernel_spmd(nc, in_maps, list(range(NCORES)))
    return reduce_outputs(res.results)



# revision 39
# speedup vs baseline: 1.0043x; 1.0043x over previous
"""CenterLoss kernel for 8 Trainium2 NeuronCores.

loss = mean(distmat * onehot(labels)) over a (B, C) distmat where
distmat[i, j] = ||x_i - c_j||^2.  The mask selects exactly one element
per row, so  loss = (1/(B*C)) * sum_i ||x_i - c_{labels[i]}||^2.

Strategy: data-parallel over batch.  Each of the 8 cores takes 512 rows
of x, gathers its 512 center rows from the (replicated) centers table
with 4 indirect DMAs (one per 128-row chunk, pipelined against the
vector engine), computes sum((x-g)^2) per chunk via subtract +
fused square-reduce (scalar_tensor_tensor accum), and writes a [128,4]
partial-sum tile.  The host sums the partials in float64 and divides
by B*C.

Raw Bass (no Tile).  Cross-engine deps are taken with
instruction-ATTACHED semaphore waits (at most one per instruction):
an attached wait parks the decoded instruction at the sequencer and
fires within ~25-310ns of the semaphore, where a standalone wait_ge
followed by a DMA_INDIRECT costs ~0.9us of GPSIMD wake/dispatch
latency (measured; the standalone->attached change alone was -0.9us).

Measured dead ends (do not revisit without new evidence): [P,2]-offset
indirect DMAs pass CoreSim but HW honors only ONE offset per partition
(rel err 0.56); warm-up indirect DMAs cost more than the dispatch
latency they hide (~1.1us fixed emission per call regardless of
descriptor count); Q7 library ops (dma_gather/ap_gather) execute in
~0.4us but the library load is ~14us on every execution; CCE
compute_op=add inflates emission 1.15->1.78us per gather; SWDGE static
loads have ~3.2us receipts vs HWDGE ~1.4us.
"""

import sys

if "/opt/trn_rl_repo" not in sys.path:
    sys.path.insert(0, "/opt/trn_rl_repo")

import numpy as np

import concourse.bass as bass
from concourse import mybir

NCORES = 8
B = 4096
D = 128
C = 20000
P = 128
BS = B // NCORES          # 512 rows per core
N = BS // P               # 4 rows per partition


def build_bass() -> bass.Bass:
    nc = bass.Bass(num_swdge_queues=2)
    x = nc.declare_dram_parameter("x", [BS, D], mybir.dt.float32, isOutput=False)
    idx = nc.declare_dram_parameter("idx", [BS], mybir.dt.int32, isOutput=False)
    centers = nc.declare_dram_parameter(
        "centers", [C, D], mybir.dt.float32, isOutput=False
    )
    out = nc.declare_dram_parameter("out", [P, N + 2], mybir.dt.float32, isOutput=True)

    with (
        nc.sbuf_tensor([P, N], mybir.dt.int32) as idx_t,
        nc.sbuf_tensor([P, N, D], mybir.dt.float32) as x_t,
        nc.sbuf_tensor([P, N, D], mybir.dt.float32) as g_t,
        nc.sbuf_tensor([P, N, D], mybir.dt.float32) as d_t,
        nc.sbuf_tensor([P, N, D], mybir.dt.float32) as sq_t,
        nc.sbuf_tensor([P, N + 2], mybir.dt.float32) as red_t,
        nc.sbuf_tensor([P, D], mybir.dt.float32) as asq_t,
        nc.semaphore("idx_sem") as idx_sem,
        nc.semaphore("x_sem") as x_sem,
        nc.semaphore("ga_sem") as ga_sem,
        nc.semaphore("gb_sem") as gb_sem,
        nc.semaphore("gc_sem") as gc_sem,
        nc.semaphore("gd_sem") as gd_sem,
        nc.semaphore("v_sem") as v_sem,
        nc.semaphore("a_sem") as a_sem,
        nc.semaphore("done_sem") as done_sem,
    ):
        g_sems = [ga_sem, gb_sem, gc_sem, gd_sem]

        # Issue the input loads in `main`, before the Block bodies: they
        # start earlier and their completion overlaps the block entry
        # overhead.  (Kept: the Block-end barrier is load-bearing — it
        # keeps the NRT per-engine postamble from contending with
        # in-flight gather completion semaphores.)
        idx_dma = nc.sync.dma_start(
            out=idx_t[:], in_=idx[:].rearrange("(p n) -> p n", p=P)
        )
        # single_packet measured inert for 128-partition transfers
        # (walrus falls back above the per-packet descriptor limit);
        # kept because it is harmless and correctness-verified.
        idx_dma.ins.single_packet = True
        idx_dma.then_inc(idx_sem, 16)
        nc.sync.dma_start(
            out=x_t[:], in_=x[:].rearrange("(p n) d -> p n d", p=P)
        ).then_inc(x_sem, 16)

        with nc.Block(no_gpsimd_drain=True) as block:

            @block.sync
            def _(sync):
                # No wait on done_sem: the Sync queue drain at block end
                # guarantees the store lands before kernel completion.
                # The ACT accumulator flush (a_sem) is the LAST gate
                # (ACTIVATE 401ns + flush 279ns vs DVE 291+83), so it
                # gets the attached wait; the earlier DVE gate takes the
                # standalone wait whose resume latency is then hidden.
                sync.wait_ge(v_sem, 2 * N)
                out_dma = sync.dma_start(out=out[:], in_=red_t[:])
                out_dma._wait_ge(a_sem, 1)
                out_dma.ins.single_packet = True
                out_dma.then_inc(done_sem, 16)

            @block.gpsimd
            def _(gpsimd):
                # HW honors only one offset per partition per indirect
                # DMA, so issue N gathers with [P, 1] offset tiles.
                # The idx wait is ATTACHED to the first gather (one
                # attached wait per instruction is allowed): a
                # standalone wait_ge retires in ~20ns but the next
                # DMA_INDIRECT slice still starts ~0.9us later (Q7
                # wake/dispatch); an attached wait holds the decoded
                # instruction at the sequencer instead.
                for n in range(N):
                    gi = gpsimd.indirect_dma_start(
                        out=g_t[:, n, :],
                        out_offset=None,
                        in_=centers[:],
                        in_offset=bass.IndirectOffsetOnAxis(
                            ap=idx_t[:, n : n + 1], axis=0
                        ),
                    )
                    if n == 0:
                        gi._wait_ge(idx_sem, 16)
                    # alternate the two SWDGE queues so transfers overlap
                    # (single_packet on the gathers measured neutral-to-
                    # slightly-worse: +45ns emission, +64ns land)
                    if n % 2 == 1:
                        gi.ins.queue = "qPoolDynamic1"
                    gi.then_inc(g_sems[n], 16)

            @block.scalar
            def _(scalar):
                # Last chunk's sum(g^2) on the ACT engine, concurrent
                # with DVE's sum(x*g): the exposed tail after the last
                # gather lands shrinks from subtract+square (~0.65us
                # serial) to max(ACT square, DVE mult-accum).
                scalar.activation(
                    out=asq_t[:],
                    in_=g_t[:, N - 1, :],
                    func=mybir.ActivationFunctionType.Square,
                    accum_out=red_t[:, N + 1 : N + 2],
                )._wait_ge(g_sems[N - 1], 16).then_inc(a_sem, 1)

            @block.vector
            def _(vector):
                vector.wait_ge(x_sem, 16)
                # Precompute sum(x^2) of the last chunk during the idle
                # window before any gather lands (free).
                vector.scalar_tensor_tensor(
                    out=sq_t[:, N - 1, :],
                    in0=x_t[:, N - 1, :],
                    scalar=0.0,
                    in1=x_t[:, N - 1, :],
                    op0=mybir.AluOpType.add,
                    op1=mybir.AluOpType.mult,
                    accum_out=red_t[:, N : N + 1],
                ).then_inc(v_sem, 1)
                # Chunk n computes while chunk n+1's gather is in
                # flight.  All waits are instruction-attached (decoded
                # op parked at the sequencer fires on the sem) — a
                # standalone wait adds ~0.1us of resume latency per
                # retirement, which is exposed on the last chunk.  The
                # v_sem chain between dependent DVE ops keeps the race
                # detector happy.
                for n in range(N - 1):
                    vector.tensor_tensor(
                        out=d_t[:, n, :],
                        in0=x_t[:, n, :],
                        in1=g_t[:, n, :],
                        op=mybir.AluOpType.subtract,
                    )._wait_ge(g_sems[n], 16).then_inc(v_sem, 1)
                    # sq = (d + 0) * d ; accum = sum(sq) — fused
                    # square+reduce
                    vector.scalar_tensor_tensor(
                        out=sq_t[:, n, :],
                        in0=d_t[:, n, :],
                        scalar=0.0,
                        in1=d_t[:, n, :],
                        op0=mybir.AluOpType.add,
                        op1=mybir.AluOpType.mult,
                        accum_out=red_t[:, n : n + 1],
                    )._wait_ge(v_sem, 2 * n + 2).then_inc(v_sem, 1)
                # last chunk: sum(x*g) only — sum(x^2) was precomputed
                # and sum(g^2) runs on ACT.  Host combines.
                vector.scalar_tensor_tensor(
                    out=d_t[:, N - 1, :],
                    in0=g_t[:, N - 1, :],
                    scalar=1.0,
                    in1=x_t[:, N - 1, :],
                    op0=mybir.AluOpType.mult,
                    op1=mybir.AluOpType.mult,
                    accum_out=red_t[:, N - 1 : N],
                )._wait_ge(g_sems[N - 1], 16).then_inc(v_sem, 1)

    if not nc.is_finalized():
        nc.finalize()
    _hoist_idx_dma(nc)
    return nc


def _hoist_idx_dma(nc: bass.Bass) -> None:
    """Move the idx DMA above the Sync engine's preamble exit barrier.

    The Sync engine finishes its own preamble work ~0.6us before the
    all-engine barrier releases (GPSIMD's const memsets are the
    laggard).  Emitting the idx load in that idle window — after Sync's
    preamble DRAIN (which must not wait on an in-flight DMA) but before
    its barrier wait — lands idx ~0.55us earlier and shifts the whole
    gather chain left.  The DMA reads DRAM (host-written before launch)
    and writes idx_t, which nothing in the preamble touches.
    """
    eb = nc.main_func.blocks[0]
    ins_list = list(eb.instructions)
    dma_i = next(
        i
        for i, x in enumerate(ins_list)
        if type(x).__name__ == "InstDMACopy"
        and str(getattr(x, "engine", "")).endswith("SP")
    )
    # Insert before Sync's first preamble register MOVE: the emission
    # then starts ~0.33us earlier still.  The preamble-exit DRAIN now
    # blocks on the idx receipt (~8.2us) and delays the all-engine
    # barrier release — but every consumer is semaphore-parked, so the
    # only effect is idx landing earlier.
    mov_i = next(
        i
        for i, x in enumerate(ins_list)
        if type(x).__name__ == "InstRegisterMove"
        and str(getattr(x, "engine", "")).endswith("SP")
    )
    assert mov_i < dma_i, (mov_i, dma_i)
    inst = ins_list.pop(dma_i)
    ins_list.insert(mov_i, inst)
    eb.instructions = ins_list


_NC = None


def _get_nc() -> bass.Bass:
    global _NC
    if _NC is None:
        _NC = build_bass()
    return _NC


def make_in_maps(x, labels, centers):
    x = np.ascontiguousarray(np.asarray(x, dtype=np.float32))
    labels = np.asarray(labels).astype(np.int32)
    centers = np.ascontiguousarray(np.asarray(centers, dtype=np.float32))
    in_maps = []
    r = np.arange(BS)
    dev_pos = (r % P) * N + (r // P)  # sorted rank r -> device row index
    for c in range(NCORES):
        sl = slice(c * BS, (c + 1) * BS)
        xs, ls = x[sl], labels[sl]
        # Sort rows by label (the loss is a sum — row order is free) and
        # place sorted rank r at device row (r%P)*N + r//P, so gather
        # chunk n reads 128 ASCENDING center addresses spanning only the
        # n-th quarter of the sorted label range: HBM-controller-friendly
        # access order instead of 128 random rows per indirect DMA.
        order = np.argsort(ls, kind="stable")
        dev = np.empty(BS, dtype=np.int64)
        dev[dev_pos] = order
        in_maps.append(
            {
                "x": np.ascontiguousarray(xs[dev]),
                "idx": np.ascontiguousarray(ls[dev]),
                "centers": centers,
            }
        )
    return in_maps


def reduce_outputs(results) -> np.ndarray:
    total = 0.0
    for r in results:
        o = r["out"].astype(np.float64)  # [P, N+2]
        # cols 0..N-2: per-chunk sum((x-g)^2); col N-1: sum(x*g) of the
        # last chunk; col N: sum(x^2); col N+1: sum(g^2)
        total += float(
            o[:, : N - 1].sum()
            + o[:, N].sum()
            + o[:, N + 1].sum()
            - 2.0 * o[:, N - 1].sum()
        )
    return np.array(np.float32(total / (B * C)))


def kernel(x, labels, centers) -> np.ndarray:
    from concourse.bass_utils import run_bass_kernel_spmd

    nc = _get_nc()
    in_maps = make_in_maps(x, labels, centers)
    res = run_bass_kernel_spmd(nc, in_maps, list(range(NCORES)))
    return reduce_outputs(res.results)

